# revision 3
# baseline (speedup 1.0000x reference)
"""GQA attention layer (B=2, S=2048, D=4096, 32 Q heads / 8 KV heads, HD=128)
with rotary embeddings, causal mask, and output projection, on 8 trn2 cores.

Sharding: tensor-parallel over heads for QKV+attention (core c owns Q heads
[4c,4c+4) and KV head c), two AllToAlls (split by head-pair, overlapped with
compute) to re-shard the attention output from head-sharded to token-sharded,
then token-sharded output projection with the full wo. Host gathers the 8
token shards.

v2 pipeline (vs the f32r baseline):
  - all matmul operands in bf16 (rate 1.0 at any free width, halves DMA/SBUF);
    PSUM accumulation stays f32. rel-err budget ~1e-2 vs the 2e-2 gate.
  - qkvT and v live in SBUF across phases (no DRAM round trip): rope results
    are written straight into persistent [128, TOK] tiles.
  - softmax denominators via "flipped" 1-column reduce matmuls (at as the
    stationary operand, ones as the 1-wide moving operand): PSUM-accumulated
    [q,4] per q-tile at ~1 cycle per block instead of a w-wide matmul.
    The [q-part,nj] -> broadcast flip runs as a PE transpose + selector
    matmuls (no DRAM bounce).
  - attention units (b, head, q-tile) are emitted interleaved into phase 1's
    nt loop as their causal k-range completes, so the exp work on ACT hides
    under projection matmuls on PE.
  - output projection in 2 passes (head-pair halves of the contraction) with
    SBUF partial staging, so pass A overlaps A2A#2 and the tail attention
    units; per-pass wo halves stream from HBM in bf16.
"""
import sys

sys.path.insert(0, "/opt/trn_rl_repo")

import numpy as np

B, S, D = 2, 2048, 4096
NH, NL, HD = 32, 8, 128
CORES = 8
QH = NH // CORES          # 4 q heads per core
TOK = B * S               # 4096
TPC = TOK // CORES        # 512 tokens per core (output sharding)
NT = 256                  # phase-1 token block width
KB_D = D // 128           # 32 contraction blocks over D
QT_W = 512                # q tile width
N_QT = S // QT_W          # 4 q tiles per batch
N_KB = S // 128           # 16 k blocks per batch
WO_NT = 256               # phase-4 dout block width
NROW = (QH + 2) * HD      # 768 qkv rows per core
NM = NROW // 128          # 6 m tiles (0..3 q heads, 4 kT, 5 vT)
NNT = TOK // NT           # 16 phase-1 token blocks
SCALE = 1.0 / np.sqrt(np.float32(HD))

_CACHE = {}


def _build_nc(mode, c_sub, sim=False):
    """mode: 'causal' | 'full' | 'generic'. c_sub: global softmax shift.
    sim=True: single-core TimelineSim variant (collective replaced by DMAs)."""
    import concourse.bacc as bacc
    import concourse.mybir as mybir
    import concourse.tile as tile
    from contextlib import ExitStack

    F32 = mybir.dt.float32
    BF = mybir.dt.bfloat16
    AT = mybir.ActivationFunctionType
    OP = mybir.AluOpType

    nc = bacc.Bacc("TRN2", target_bir_lowering=False, debug=False,
                   num_devices=1 if sim else CORES)

    xT_d = nc.dram_tensor("xT", (D, TOK), BF, kind="ExternalInput").ap()
    wqkvT_d = nc.dram_tensor("wqkvT", (D, NROW), BF, kind="ExternalInput").ap()
    woT_d = nc.dram_tensor("woT", (D, D), BF, kind="ExternalInput").ap()
    cosP_d = nc.dram_tensor("cosP", (HD, TOK), BF, kind="ExternalInput").ap()
    sinP_d = nc.dram_tensor("sinP", (HD, TOK), BF, kind="ExternalInput").ap()
    if mode == "generic":
        biasT_d = nc.dram_tensor("biasT", (S, S), F32, kind="ExternalInput").ap()
    out_d = nc.dram_tensor("out", (TPC, D), F32, kind="ExternalOutput").ap()

    ident_h = nc.inline_tensor(np.eye(128, dtype=np.float32), name="ident")
    pswap = np.zeros((128, 128), dtype=np.float32)
    for i in range(64):
        pswap[2 * i, 2 * i + 1] = -1.0
        pswap[2 * i + 1, 2 * i] = 1.0
    pswapT_h = nc.inline_tensor(np.ascontiguousarray(pswap.T), name="pswapT")
    # within-diag-block causal mask: keep q-local >= k-local
    tri = np.zeros((128, 128), dtype=np.float32)
    for p in range(128):
        tri[p, p:] = 1.0
    tri_h = nc.inline_tensor(tri, name="trimask")
    ones_col_h = nc.inline_tensor(np.ones((128, 1), np.float32), name="ones_col")
    esel = np.zeros((4, 4 * 128), dtype=np.float32)
    for j in range(4):
        esel[j, 128 * j:128 * (j + 1)] = 1.0
    esel_h = nc.inline_tensor(esel, name="esel")

    with tile.TileContext(nc) as tc, ExitStack() as glob:
        dram = glob.enter_context(tc.tile_pool(name="dram", bufs=1, space="DRAM"))
        consts = glob.enter_context(tc.tile_pool(name="consts", bufs=1))
        persist = glob.enter_context(tc.tile_pool(name="persist", bufs=1))

        # split A2A: hp=0 carries head-locals {0,1}, hp=1 carries {2,3}
        a2a_in = [dram.tile([TOK // 2, TPC], BF, name=f"a2a_in{hp}")
                  for hp in range(2)]
        a2a_out = [dram.tile([TOK // 2, TPC], BF, name=f"a2a_out{hp}")
                   for hp in range(2)]

        def cast_const(h, shape, name):
            t32 = consts.tile(shape, F32, name=name + "32")
            nc.sync.dma_start(t32[:], h.ap())
            t = consts.tile(shape, BF, name=name)
            nc.vector.tensor_copy(t[:], t32[:])
            return t

        consts_loaded = {}

        def get_consts():
            # emitted after the first weight DMAs so the tiny const loads do
            # not delay the critical startup path
            if not consts_loaded:
                consts_loaded["ident"] = cast_const(ident_h, [128, 128], "ident")
                consts_loaded["pswapT"] = cast_const(pswapT_h, [128, 128],
                                                     "pswapT")
                consts_loaded["tri"] = cast_const(tri_h, [128, 128], "tri")
                consts_loaded["ones_col"] = cast_const(ones_col_h, [128, 1],
                                                       "ones_col")
                consts_loaded["esel"] = cast_const(esel_h, [4, 4 * 128], "esel")
            return consts_loaded

        # persistent SBUF tensors: roped q heads + kT in [chan, tok] layout,
        # v in natural [tok, chan] layout (4 token-blocks per tile).
        qkvT_sb = [persist.tile([128, TOK], BF, name=f"qkvT{m}")
                   for m in range(NM - 1)]
        v_sb = [persist.tile([128, 4 * HD], BF, name=f"v{g}")
                for g in range(NNT // 2)]

        def v_view(b, kb):
            g = N_KB * b + kb  # global 128-token block index
            return v_sb[g // 4][:, HD * (g % 4):HD * (g % 4) + HD]

        # ---------------- pools (p2 pools open before the phase-1 stack so
        # pool release order stays LIFO)
        p2_psc = glob.enter_context(tc.tile_pool(name="p2_psc", bufs=2,
                                                 space="PSUM"))
        p2_pyq = glob.enter_context(tc.tile_pool(name="p2_pyq", bufs=2,
                                                 space="PSUM"))
        p2_pr = glob.enter_context(tc.tile_pool(name="p2_pr", bufs=1,
                                                space="PSUM"))
        p2_at = glob.enter_context(tc.tile_pool(name="p2_at", bufs=6))
        p2_ms = glob.enter_context(tc.tile_pool(name="p2_ms", bufs=3))
        if mode == "generic":
            p2_bias = glob.enter_context(tc.tile_pool(name="p2_bias", bufs=4))

        kb_max_of = (lambda qt: 4 * qt + 4) if mode == "causal" else (lambda qt: N_KB)
        CONSTS = {}

        # ============ phase 2 unit machinery (emitted interleaved) ==========
        # a unit covers q columns [q0, q0+qw) of q-tile qt for (b, h); the
        # tail units split qt3 in half so the first half's exp hides under
        # the last phase-1 block.
        p2_open = {}

        def p2_open_unit(b, h, qt, p2_pyq, q0=0, qw=QT_W):
            py = p2_pyq.tile([128, QT_W], F32, name="pyq")
            # per-(j, kb) partial denominator columns; every flip matmul is an
            # independent start+stop group (interleaved open accumulation
            # groups in one PSUM tile corrupt each other)
            psq = p2_pyq.tile([128, 4 * N_KB], F32, name="pyq")
            p2_open[(b, h, qt, q0)] = (py, psq)

        def p2_emit_kbs(b, h, qt, kb_lo, kb_hi, p2_psc, q0=0, qw=QT_W):
            py, psq = p2_open[(b, h, qt, q0)]
            t0 = S * b
            qbase = (QT_W * qt + q0) // 128      # first q 128-slice index
            nj = qw // 128
            kb_max = qbase + nj if mode == "causal" else N_KB
            kT = qkvT_sb[QH]
            qT = qkvT_sb[h]
            gq0 = t0 + QT_W * qt + q0
            for kb in range(kb_lo, min(kb_hi, kb_max)):
                diag_j = kb - qbase if (mode == "causal" and kb >= qbase) else -1
                c0 = 128 * diag_j if diag_j > 0 else 0
                psc = p2_psc.tile([128, QT_W], F32, name="psc")
                nc.tensor.matmul(
                    psc[:, c0:qw], kT[:, t0 + 128 * kb:t0 + 128 * (kb + 1)],
                    qT[:, gq0 + c0:gq0 + qw],
                    start=True, stop=True)
                if mode == "generic":
                    bt = p2_bias.tile([128, QT_W], F32, name="bt")
                    nc.sync.dma_start(
                        bt[:, :qw], biasT_d[128 * kb:128 * (kb + 1),
                                            QT_W * qt + q0:QT_W * qt + q0 + qw])
                    nc.vector.tensor_tensor(psc[:, :qw], psc[:, :qw],
                                            bt[:, :qw], op=OP.add)
                at = p2_at.tile([128, QT_W], BF, name="at")
                nc.scalar.activation(at[:, c0:qw], psc[:, c0:qw], AT.Exp,
                                     bias=-float(c_sub), scale=float(SCALE))
                if diag_j >= 0:
                    nc.vector.tensor_tensor(
                        at[:, c0:c0 + 128], at[:, c0:c0 + 128], CONSTS["tri"][:],
                        op=OP.mult)
                nc.tensor.matmul(py[:, c0:qw], v_view(b, kb), at[:, c0:qw],
                                 start=(kb == 0), stop=(kb == kb_max - 1),
                                 skip_group_check=True)
                # flipped denominator reduce: 1-wide moving operand, one
                # independent start+stop matmul per (q-slice, kb) column
                for j in range(max(diag_j, 0), nj):
                    nc.tensor.matmul(psq[:, N_KB * j + kb:N_KB * j + kb + 1],
                                     at[:, 128 * j:128 * (j + 1)],
                                     CONSTS["ones_col"][:],
                                     start=True, stop=True,
                                     skip_group_check=True)

        def p2_finalize(b, h, qt, p2_pr, q0=0, qw=QT_W):
            py, psq = p2_open.pop((b, h, qt, q0))
            hp = h // 2
            qbase = (QT_W * qt + q0) // 128
            nj = qw // 128
            kb_max = qbase + nj if mode == "causal" else N_KB
            sq = p2_ms.tile([128, 4], F32, name="sq")
            for j in range(nj):
                nv = min(qbase + j + 1, kb_max) if mode == "causal" else kb_max
                nc.vector.tensor_reduce(
                    sq[:, j:j + 1], psq[:, N_KB * j:N_KB * j + nv],
                    axis=mybir.AxisListType.X, op=OP.add)
            with nc.allow_low_precision(reason="bf16 softmax denominators"):
                recq = p2_ms.tile([128, 4], BF, name="recq")
                nc.vector.reciprocal(recq[:, :nj], sq[:, :nj])
            # partition->free flip via PE transpose, then per-slice selector
            # matmuls broadcast 1/denom across the 128 channel partitions
            pt4 = p2_pr.tile([4, 128], BF, name="pr")
            nc.tensor.transpose(pt4[0:nj, :], recq[:, :nj], CONSTS["ident"][:])
            rt = p2_ms.tile([4, 128], BF, name="rt")
            nc.vector.tensor_copy(rt[0:nj, :], pt4[0:nj, :])
            pr = p2_pr.tile([128, QT_W], F32, name="pr")
            for j in range(nj):
                nc.tensor.matmul(pr[:, 128 * j:128 * (j + 1)],
                                 CONSTS["esel"][0:nj, 128 * j:128 * (j + 1)],
                                 rt[0:nj, :], start=True, stop=True)
            rep = p2_ms.tile([128, QT_W], BF, name="rep")
            nc.vector.tensor_copy(rep[:, :qw], pr[:, :qw])
            yT = p2_ms.tile([128, QT_W], BF, name="yT")
            nc.vector.tensor_tensor(yT[:, :qw], py[:, :qw], rep[:, :qw],
                                    op=OP.mult)
            j = 4 * b + qt
            r0 = (TPC // 2) * j + 128 * (h % 2)
            nc.sync.dma_start(a2a_in[hp][r0:r0 + 128, q0:q0 + qw],
                              yT[:, :qw])

        def p2_full_unit(b, h, qt, p2_psc, p2_pyq, p2_pr, q0=0, qw=QT_W):
            p2_open_unit(b, h, qt, p2_pyq, q0, qw)
            p2_emit_kbs(b, h, qt, 0, N_KB, p2_psc, q0, qw)
            p2_finalize(b, h, qt, p2_pr, q0, qw)

        def do_a2a(hp):
            if sim:
                for j in range(CORES):
                    nc.sync.dma_start(
                        a2a_out[hp][(TPC // 2) * j:(TPC // 2) * (j + 1), :],
                        a2a_in[hp][(TPC // 2) * j:(TPC // 2) * (j + 1), :])
            else:
                nc.gpsimd.collective_compute(
                    "AllToAll", mybir.AluOpType.bypass,
                    replica_groups=[list(range(CORES))],
                    ins=[a2a_in[hp][:]], outs=[a2a_out[hp][:]],
                )

        # ================= phase 1 + interleaved phase-2 units ==============
        with ExitStack() as ctx1:
            p1_w = ctx1.enter_context(tc.tile_pool(name="p1_w", bufs=1))
            p1_x = ctx1.enter_context(tc.tile_pool(name="p1_x", bufs=2))
            p1_cs = ctx1.enter_context(tc.tile_pool(name="p1_cs", bufs=2))
            p1_st = ctx1.enter_context(tc.tile_pool(name="p1_st", bufs=3))
            p1_pa = ctx1.enter_context(tc.tile_pool(name="p1_pa", bufs=2,
                                                    space="PSUM"))
            p1_pb = ctx1.enter_context(tc.tile_pool(name="p1_pb", bufs=1,
                                                    space="PSUM"))

            def load_x(c0, ntw, split=False):
                xts = []
                for i in range(2):
                    xt = p1_x.tile([128, 16 * ntw], BF, name=f"x{i}")
                    if split:
                        # sub-DMAs so the first chains start while the rest
                        # of the block streams in
                        for qq in range(4):
                            nc.sync.dma_start(
                                xt[:].rearrange("p (g c) -> p g c", g=16)
                                [:, 4 * qq:4 * (qq + 1), :],
                                xT_d[2048 * i + 512 * qq:
                                     2048 * i + 512 * (qq + 1), c0:c0 + ntw]
                                .rearrange("(g p) c -> p g c", p=128))
                    else:
                        nc.sync.dma_start(
                            xt[:].rearrange("p (g c) -> p g c", g=16),
                            xT_d[2048 * i:2048 * (i + 1), c0:c0 + ntw]
                            .rearrange("(g p) c -> p g c", p=128))
                    xts.append(xt)
                cos_sb = p1_cs.tile([128, ntw], BF, name="cos_sb")
                nc.sync.dma_start(cos_sb[:], cosP_d[:, c0:c0 + ntw])
                sin_sb = p1_cs.tile([128, ntw], BF, name="sin_sb")
                nc.sync.dma_start(sin_sb[:], sinP_d[:, c0:c0 + ntw])
                return xts, cos_sb, sin_sb

            # token blocks: a 512-wide first block (PE consumption matches
            # the initial weight-DMA arrival rate), then 256-wide blocks
            blocks = [(0, 512)] + [(512 + 256 * k, 256) for k in range(14)]

            # interleave schedule: after finishing block bi, emit these p2
            # units (b, h, qt); readiness follows cumulative token coverage
            sched = {}
            if mode == "causal":
                for qt in range(4):
                    sched[2 * qt] = [(0, 0, qt), (0, 1, qt)]
                    sched[2 * qt + 1] = [(0, 2, qt), (0, 3, qt)]
                for qt in range(3):
                    sched[8 + 2 * qt] = [(1, 0, qt), (1, 1, qt)]
                    sched[9 + 2 * qt] = [(1, 2, qt), (1, 3, qt)]
            else:
                pairs = [(0, hh, qt) for hh in range(4) for qt in range(4)]
                for i in range(8):
                    sched[6 + i] = pairs[2 * i:2 * i + 2]

            # block-0 x and qkv-weight DMAs interleaved in the order the
            # first chains consume them (kb-major), so PE starts at ~4us
            xts = [p1_x.tile([128, 16 * 512], BF, name=f"x{i}")
                   for i in range(2)]

            def x_sub(i, qq):
                nc.sync.dma_start(
                    xts[i][:].rearrange("p (g c) -> p g c", g=16)
                    [:, 4 * qq:4 * (qq + 1), :],
                    xT_d[2048 * i + 512 * qq:2048 * i + 512 * (qq + 1), 0:512]
                    .rearrange("(g p) c -> p g c", p=128))

            w_sb = [p1_w.tile([128, 2 * NROW], BF, name=f"w{i}")
                    for i in range(16)]

            def w_dma(i):
                nc.sync.dma_start(
                    w_sb[i][:].rearrange("p (g c) -> p g c", g=2),
                    wqkvT_d[256 * i:256 * (i + 1), :]
                    .rearrange("(g p) c -> p g c", p=128))

            x_sub(0, 0)
            w_dma(0)
            w_dma(1)
            CONSTS.update(get_consts())
            cos_sb = p1_cs.tile([128, 512], BF, name="cos_sb")
            nc.sync.dma_start(cos_sb[:], cosP_d[:, 0:512])
            sin_sb = p1_cs.tile([128, 512], BF, name="sin_sb")
            nc.sync.dma_start(sin_sb[:], sinP_d[:, 0:512])
            for step in range(7):
                i, qq = divmod(step + 1, 4)
                x_sub(i, qq)
                w_dma(2 * step + 2)
                w_dma(2 * step + 3)

            def wv(kb):
                return w_sb[kb // 2][:, NROW * (kb % 2):NROW * (kb % 2) + NROW]

            # during block 0 the attention pools are still idle: borrow
            # their PSUM slots so 5 qkv chains run concurrently and consume
            # the arriving weight tiles at full DMA rate
            b0_pools = [(p1_pa, "pamm"), (p1_pa, "pamm"), (p1_pb, "pbmm"),
                        (p2_psc, "psc"), (p2_psc, "psc"), (p1_pa, "pamm")]

            for bi, (c0, ntw) in enumerate(blocks):
                if bi + 1 < len(blocks):
                    nxts = load_x(blocks[bi + 1][0], blocks[bi + 1][1])
                for m in range(NM):
                    if bi == 0:
                        pool_m, name_m = b0_pools[m]
                        pa = pool_m.tile([128, ntw], F32, name=name_m)
                    else:
                        pa = p1_pa.tile([128, ntw], F32, name="pamm")
                    for kb in range(KB_D):
                        nc.tensor.matmul(
                            pa[:], wv(kb)[:, 128 * m:128 * (m + 1)],
                            xts[kb // 16][:, ntw * (kb % 16):ntw * (kb % 16) + ntw],
                            start=(kb == 0), stop=(kb == KB_D - 1))
                    if m < NM - 1:
                        # rope: pair-swap via PE, cos/sin scaling on DVE,
                        # result written straight into the persistent tile
                        a_sb = p1_st.tile([128, ntw], BF, name="a_sb")
                        nc.scalar.copy(a_sb[:], pa[:])
                        pb = p1_pb.tile([128, ntw], F32, name="pbmm")
                        nc.tensor.matmul(pb[:], CONSTS["pswapT"][:], a_sb[:],
                                         start=True, stop=True)
                        tcos = p1_st.tile([128, ntw], F32, name="tcos")
                        nc.vector.tensor_tensor(tcos[:], pa[:], cos_sb[:],
                                                op=OP.mult)
                        tsin = p1_st.tile([128, ntw], F32, name="tsin")
                        nc.vector.tensor_tensor(tsin[:], pb[:], sin_sb[:],
                                                op=OP.mult)
                        nc.vector.tensor_tensor(qkvT_sb[m][:, c0:c0 + ntw],
                                                tcos[:], tsin[:], op=OP.add)
                    else:
                        # vT -> v natural via PE transposes into persistent v
                        vst = p1_st.tile([128, ntw], BF, name="vst")
                        nc.scalar.copy(vst[:], pa[:])
                        for jj in range(ntw // 128):
                            g = c0 // 128 + jj
                            pool = p1_pb if jj % 2 == 0 else p1_pa
                            pt = pool.tile([128, 128], BF,
                                           name="pbmm" if jj % 2 == 0 else "pamm")
                            nc.tensor.transpose(
                                pt[:], vst[:, 128 * jj:128 * (jj + 1)],
                                CONSTS["ident"][:])
                            nc.vector.tensor_copy(
                                v_sb[g // 4][:, HD * (g % 4):HD * (g % 4) + HD],
                                pt[:])
                if bi + 1 < len(blocks):
                    xts, cos_sb, sin_sb = nxts
                for (ub, uh, uqt) in sched.get(bi, ()):
                    p2_full_unit(ub, uh, uqt, p2_psc, p2_pyq, p2_pr)
                if mode == "causal" and bi == 13:
                    # first halves of the b1 qt3 h0/h1 units (the A2A#1
                    # critical path): ready here, exp hides under the last
                    # two blocks
                    for hh in range(2):
                        p2_full_unit(1, hh, 3, p2_psc, p2_pyq, p2_pr,
                                     q0=0, qw=QT_W // 2)

        # ================= tail attention units + A2As + phase 4 ============
        p4_w = glob.enter_context(tc.tile_pool(name="p4_w", bufs=3))
        p4_y = glob.enter_context(tc.tile_pool(name="p4_y", bufs=1))
        p4_part = glob.enter_context(tc.tile_pool(name="p4_part", bufs=1))
        p4_o = glob.enter_context(tc.tile_pool(name="p4_o", bufs=4))

        with ExitStack() as ctxT:
            pyq_t = ctxT.enter_context(tc.tile_pool(name="pyq_t", bufs=2,
                                                    space="PSUM"))
            if mode == "causal":
                p2_full_unit(1, 0, 3, p2_psc, p2_pyq, p2_pr,
                             q0=QT_W // 2, qw=QT_W // 2)
                p2_full_unit(1, 1, 3, p2_psc, pyq_t, p2_pr,
                             q0=QT_W // 2, qw=QT_W // 2)
            else:
                for n, (qt, hh) in enumerate([(qt, hh) for qt in range(4)
                                              for hh in range(2)]):
                    p2_full_unit(1, hh, qt, p2_psc,
                                 p2_pyq if n % 2 == 0 else pyq_t, p2_pr)
            do_a2a(0)

        with ExitStack() as ctx4:
            p4_po = ctx4.enter_context(tc.tile_pool(name="p4_po", bufs=3,
                                                    space="PSUM"))

            # phase-4 helpers. channel block kb32 = 4*i + hl (src core i,
            # head-local hl); pass A covers hl in {0,1}, pass B {2,3}.
            def pass_kbs(hp):
                return [4 * i + hl for i in range(8)
                        for hl in (2 * hp, 2 * hp + 1)]

            y_tiles = {}

            def load_y(hp):
                for kb32 in pass_kbs(hp):
                    i, hl = kb32 // 4, kb32 % 4
                    r0 = 256 * i + 128 * (hl % 2)
                    yt = p4_y.tile([128, TPC], BF, name=f"y{kb32}")
                    nc.sync.dma_start(yt[:], a2a_out[hl // 2][r0:r0 + 128, :])
                    y_tiles[kb32] = yt

            def load_wo(hp, do):
                # only this pass's half of the wo rows (head-pair channels);
                # one DMA per head-local to keep each AP at 3 dims
                wt = p4_w.tile([128, 16 * WO_NT], BF, name="wo")
                dst = wt[:].rearrange("p (i hl c) -> p hl i c", i=8, hl=2)
                src = woT_d[:, WO_NT * do:WO_NT * (do + 1)].rearrange(
                    "(i f p) c -> p f i c", i=8, f=4, p=128)
                for hl in range(2):
                    nc.sync.dma_start(dst[:, hl], src[:, 2 * hp + hl])
                return wt

            def wov(wt, kb32):
                i, hl = kb32 // 4, kb32 % 4
                n = 2 * i + (hl % 2)
                return wt[:, WO_NT * n:WO_NT * (n + 1)]

            part_tiles = [p4_part.tile([128, 4 * WO_NT], F32, name=f"part{do}")
                          for do in range(D // WO_NT)]

            # pass A (hp0 channels), interleaved with the remaining tail units
            load_y(0)
            kbsA = pass_kbs(0)
            for do in range(D // WO_NT):
                wt = load_wo(0, do)
                for tb in range(TPC // 128):
                    po = p4_po.tile([128, WO_NT], F32, name="po")
                    for n, kb32 in enumerate(kbsA):
                        nc.tensor.matmul(po[:],
                                         y_tiles[kb32][:, 128 * tb:128 * (tb + 1)],
                                         wov(wt, kb32), start=(n == 0),
                                         stop=(n == 15))
                    nc.vector.tensor_copy(
                        part_tiles[do][:, WO_NT * tb:WO_NT * (tb + 1)], po[:])
                if mode == "causal":
                    if do == 0:
                        p2_full_unit(1, 2, 3, p2_psc, p2_pyq, p2_pr)
                    elif do == 1:
                        p2_full_unit(1, 3, 3, p2_psc, p2_pyq, p2_pr)
                        do_a2a(1)
                else:
                    if do < 8:
                        qt, hh = do // 2, 2 + do % 2
                        p2_full_unit(1, hh, qt, p2_psc, p2_pyq, p2_pr)
                        if do == 7:
                            do_a2a(1)

            # pass B (hp1 channels) + combine + store
            load_y(1)
            kbsB = pass_kbs(1)
            for do in range(D // WO_NT):
                wt = load_wo(1, do)
                for tb in range(TPC // 128):
                    po = p4_po.tile([128, WO_NT], F32, name="po")
                    for n, kb32 in enumerate(kbsB):
                        nc.tensor.matmul(po[:],
                                         y_tiles[kb32][:, 128 * tb:128 * (tb + 1)],
                                         wov(wt, kb32), start=(n == 0),
                                         stop=(n == 15))
                    o_sb = p4_o.tile([128, WO_NT], F32, name="o_sb")
                    nc.vector.tensor_tensor(
                        o_sb[:], po[:],
                        part_tiles[do][:, WO_NT * tb:WO_NT * (tb + 1)],
                        op=OP.add)
                    nc.sync.dma_start(
                        out_d[128 * tb:128 * (tb + 1),
                              WO_NT * do:WO_NT * (do + 1)], o_sb[:])

    nc.compile()
    return nc


def _prepare(x, freqs_cis, mask, wqkv_w, wo_w):
    """Host-side prep: mode detection, stability constant, input maps."""
    import ml_dtypes
    BF = ml_dtypes.bfloat16

    x = np.asarray(x, dtype=np.float32)
    freqs_cis = np.asarray(freqs_cis, dtype=np.float32)
    mask = np.asarray(mask)
    wqkv_w = np.asarray(wqkv_w, dtype=np.float32)
    wo_w = np.asarray(wo_w, dtype=np.float32)

    m2 = mask.reshape(mask.shape[-2], mask.shape[-1])
    if np.array_equal(m2, np.tril(np.ones((S, S), dtype=bool))):
        mode = "causal"
    elif m2.all():
        mode = "full"
    else:
        mode = "generic"

    x2 = x.reshape(TOK, D)
    xT = np.ascontiguousarray(x2.T).astype(BF)
    woT = np.ascontiguousarray(wo_w.T).astype(BF)

    cos = freqs_cis[:, :, 0].T          # [64, S]
    sin = freqs_cis[:, :, 1].T
    cosP = np.repeat(cos, 2, axis=0)    # [128, S]
    sinP = np.repeat(sin, 2, axis=0)
    cosP = np.ascontiguousarray(np.tile(cosP, (1, B))).astype(BF)
    sinP = np.ascontiguousarray(np.tile(sinP, (1, B))).astype(BF)

    # softmax stability probe: rope'd scores for head 0, batch 0, 128 q rows
    wq0 = wqkv_w[:HD]                   # [128, D]
    wk0 = wqkv_w[NH * HD:NH * HD + HD]  # [128, D]
    qs = x2[:128] @ wq0.T               # [128, 128]
    ks = x2[:S] @ wk0.T                 # [S, 128]

    def rope_np(t, fc):
        ts = t.reshape(t.shape[0], HD // 2, 2)
        c, s_ = fc[:t.shape[0], :, 0], fc[:t.shape[0], :, 1]
        out = np.empty_like(ts)
        out[:, :, 0] = ts[:, :, 0] * c - ts[:, :, 1] * s_
        out[:, :, 1] = ts[:, :, 1] * c + ts[:, :, 0] * s_
        return out.reshape(t.shape)

    qs = rope_np(qs, freqs_cis)
    ks = rope_np(ks, freqs_cis)
    smax = float(np.max(np.abs(qs @ ks.T)) * SCALE)
    c_sub = 0.0 if smax < 25.0 else smax + 5.0

    in_maps = []
    for c in range(CORES):
        wq_c = wqkv_w[QH * HD * c:QH * HD * (c + 1)]
        wk_c = wqkv_w[NH * HD + HD * c:NH * HD + HD * (c + 1)]
        wv_c = wqkv_w[(NH + NL) * HD + HD * c:(NH + NL) * HD + HD * (c + 1)]
        wqkvT_c = np.ascontiguousarray(np.vstack([wq_c, wk_c, wv_c]).T).astype(BF)
        m = {"xT": xT, "wqkvT": wqkvT_c, "woT": woT, "cosP": cosP, "sinP": sinP}
        if mode == "generic":
            m["biasT"] = np.ascontiguousarray(
                np.where(m2.T, np.float32(0), np.float32(-1e30)))
        in_maps.append(m)
    return mode, c_sub, in_maps


def _get_nc(mode, c_sub):
    key = (mode, round(float(c_sub), 3))
    if key not in _CACHE:
        _CACHE[key] = _build_nc(mode, c_sub)
    return _CACHE[key]


def kernel(x, freqs_cis, mask, wqkv_w, wo_w):
    from concourse import bass_utils
    mode, c_sub, in_maps = _prepare(x, freqs_cis, mask, wqkv_w, wo_w)
    nc = _get_nc(mode, c_sub)
    res = bass_utils.run_bass_kernel_spmd(nc, in_maps, core_ids=list(range(CORES)))
    out = np.concatenate([res.results[c]["out"] for c in range(CORES)], axis=0)
    return out.reshape(B, S, D)
